# revision 1
# baseline (speedup 1.0000x reference)
"""Trainium2 Bass kernel for nn_AlignModule (QAConv correlation + PAM).

Reference computation (B=32, C=512, H=24, W=8, hw=192, C8=64):
  xf = x.reshape(B, C, hw)
  score[g,p,n,m] = sum_c xf[g,c,m] * xf[p,c,n]          # [B,B,hw,hw]
  kernel_max[g,p,n] = max_m score[g,p,n,m]              # [B,B,hw]
  q = Wq @ xf[b] + bq; k = Wk @ xf[b] + bk              # [B,C8,hw]
  energy[b,m,n] = sum_q q[b,q,m] k[b,q,n]
  pos_max[b,m] = max_n energy[b,m,n]                    # [B,hw]
  out = concat([kernel_max, pos_max[None]], axis=0)     # [B+1,B,hw]

Sharding: data-parallel over g (and b for PAM) across 8 cores, 4 per core.
Each core receives the full x as a [C, B*hw] matrix (xT), rolled along the
column axis so its own 4 images occupy columns [0, 4*hw). The same SPMD
program then always reads its moving operand from columns [0, 768).

All QAConv matmuls are fp32r (FP22 multiply, fp32 accumulate) with moving
free dim 384 — the full-rate regime of the PE for 4-byte operands. The
stationary side packs the flattened (p, n) axis perfectly into 48
128-column blocks, so the PE runs at its fp32 roofline for this shape.
"""

import numpy as np

import concourse.bass as bass
import concourse.mybir as mybir
import concourse.tile as tile
from concourse import bacc
from concourse.bass_utils import run_bass_kernel_spmd
from concourse.masks import make_identity

B = 32
C = 512
HW = 192
C8 = 64
N_CORES = 8
GPC = B // N_CORES            # images per core (4)
FLAT = B * HW                 # flattened (p, n) axis (6144)
NJ = FLAT // 128              # stationary 128-column blocks (48)
NCC = 4                       # contraction chunks of 128 over C
GROLL = GPC * HW              # per-core roll step (768)
N_GP = GPC // 2               # g-pairs (moving operand batches of 2*HW=384)
JPCC = 6                      # j blocks per column-chunk of 768
F32 = mybir.dt.float32
F32R = mybir.dt.float32r
AX_X = mybir.AxisListType.X

_COMPILED = None


def _build():
    nc = bacc.Bacc("TRN2", target_bir_lowering=False, debug=False)

    xr = nc.dram_tensor("xr", [C, FLAT], F32R, kind="ExternalInput").ap()
    wq = nc.dram_tensor("wq", [C, C8], F32R, kind="ExternalInput").ap()
    wk = nc.dram_tensor("wk", [C, C8], F32R, kind="ExternalInput").ap()
    bq = nc.dram_tensor("bq", [C8, 1], F32, kind="ExternalInput").ap()
    bk = nc.dram_tensor("bk", [C8, 1], F32, kind="ExternalInput").ap()
    kmax = nc.dram_tensor("kmax", [GPC, FLAT], F32, kind="ExternalOutput").ap()
    pmax = nc.dram_tensor("pmax", [GPC, HW], F32, kind="ExternalOutput").ap()

    with tile.TileContext(nc) as tc:
        with (
            tc.tile_pool(name="xpool", bufs=1) as xpool,
            tc.tile_pool(name="wpool", bufs=1) as wpool,
            tc.tile_pool(name="respool", bufs=1) as respool,
            tc.tile_pool(name="outpool", bufs=1) as outpool,
            tc.tile_pool(name="qa_psum", bufs=6, space="PSUM") as qa_psum,
            tc.tile_pool(name="pam_psum", bufs=2, space="PSUM") as pam_psum,
        ):
            # ---- weights / biases ----
            wq_sb = wpool.tile([128, NCC, C8], F32R)
            nc.sync.dma_start(wq_sb[:], wq.rearrange("(co p) q -> p co q", p=128))
            wk_sb = wpool.tile([128, NCC, C8], F32R)
            nc.sync.dma_start(wk_sb[:], wk.rearrange("(co p) q -> p co q", p=128))
            bq_sb = wpool.tile([C8, 1], F32)
            nc.sync.dma_start(bq_sb[:], bq[:])
            bk_sb = wpool.tile([C8, 1], F32)
            nc.sync.dma_start(bk_sb[:], bk[:])

            ident = wpool.tile([128, 128], F32)
            make_identity(nc, ident[:])

            # ---- load x: 4 c-chunks x 8 column chunks of 768 ----
            xcb = [[None] * (FLAT // GROLL) for _ in range(NCC)]
            for cc in range(FLAT // GROLL):
                for c in range(NCC):
                    t = xpool.tile([128, GROLL], F32R, tag=f"x_{c}_{cc}")
                    nc.sync.dma_start(
                        t[:],
                        xr[c * 128:(c + 1) * 128, cc * GROLL:(cc + 1) * GROLL],
                    )
                    xcb[c][cc] = t

            # ---- PAM: q/k projections ----
            q_sb = wpool.tile([C8, GPC * HW], F32R)
            k_sb = wpool.tile([C8, GPC * HW], F32R)
            for gp in range(N_GP):
                q_ps = pam_psum.tile([C8, 2 * HW], F32, tag="pam", name=f"q_ps_{gp}")
                k_ps = pam_psum.tile([C8, 2 * HW], F32, tag="pam", name=f"k_ps_{gp}")
                for c in range(NCC):
                    rhs = xcb[c][0][:, gp * 2 * HW:(gp + 1) * 2 * HW]
                    nc.tensor.matmul(
                        q_ps[:], wq_sb[:, c, :], rhs,
                        start=(c == 0), stop=(c == NCC - 1),
                    )
                    nc.tensor.matmul(
                        k_ps[:], wk_sb[:, c, :], rhs,
                        start=(c == 0), stop=(c == NCC - 1),
                    )
                sl = slice(gp * 2 * HW, (gp + 1) * 2 * HW)
                nc.scalar.activation(
                    q_sb[:, sl], q_ps[:],
                    mybir.ActivationFunctionType.Identity, bias=bq_sb[:],
                )
                nc.scalar.activation(
                    k_sb[:, sl], k_ps[:],
                    mybir.ActivationFunctionType.Identity, bias=bk_sb[:],
                )

            # ---- PAM: energy + max over n ----
            # pam_sb[p, b, h]: h=0 -> pos_max[b, p] (m in [0,128));
            #                  h=1, p<64 -> pos_max[b, 128+p]
            pam_sb = respool.tile([128, GPC, 2], F32)
            for b in range(GPC):
                for mch, (m0, msz) in enumerate(((0, 128), (128, C8))):
                    e_ps = pam_psum.tile([128, HW], F32, tag="pam", name=f"e_ps_{b}_{mch}")
                    nc.tensor.matmul(
                        e_ps[:msz, :],
                        q_sb[:, b * HW + m0: b * HW + m0 + msz],
                        k_sb[:, b * HW:(b + 1) * HW],
                        start=True, stop=True,
                    )
                    nc.vector.reduce_max(
                        pam_sb[:msz, b, mch:mch + 1], e_ps[:msz, :], axis=AX_X,
                    )

            pam_t = outpool.tile([2 * GPC, 128], F32)
            tp = pam_psum.tile([128, 128], F32, tag="pam", name="tp_pam")
            nc.tensor.transpose(
                tp[:2 * GPC, :],
                pam_sb[:].rearrange("p b h -> p (b h)"),
                ident[:],
            )
            nc.scalar.copy(pam_t[:], tp[:2 * GPC, :])
            for b in range(GPC):
                nc.sync.dma_start(
                    pmax[b:b + 1, 0:128], pam_t[2 * b:2 * b + 1, :]
                )
                nc.sync.dma_start(
                    pmax[b:b + 1, 128:HW], pam_t[2 * b + 1:2 * b + 2, 0:C8]
                )

            # ---- QAConv: score blocks + max over m ----
            # res_sb[p, g, j] = kernel_max[g, j*128 + p] (rolled flat order)
            res_sb = respool.tile([128, GPC, NJ], F32)
            for j in range(NJ):
                cc, jl = divmod(j, JPCC)
                ps = [qa_psum.tile([128, 2, HW], F32, tag="qa_ps",
                                   name=f"qa_ps_{j}_{gp}")
                      for gp in range(N_GP)]
                for c in range(NCC):
                    lhsT = xcb[c][cc][:, jl * 128:(jl + 1) * 128]
                    for gp in range(N_GP):
                        rhs = xcb[c][0][:, gp * 2 * HW:(gp + 1) * 2 * HW]
                        nc.tensor.matmul(
                            ps[gp][:].rearrange("p a b -> p (a b)"),
                            lhsT,
                            rhs,
                            start=(c == 0),
                            stop=(c == NCC - 1),
                        )
                for gp in range(N_GP):
                    nc.vector.reduce_max(
                        res_sb[:, 2 * gp:2 * gp + 2, j], ps[gp][:], axis=AX_X,
                    )

            # ---- transpose results to output layout and store ----
            kout = outpool.tile([128, GPC, 128], F32)
            for g in range(GPC):
                tp = pam_psum.tile([128, 128], F32, tag="pam", name=f"tp_{g}")
                nc.tensor.transpose(tp[:NJ, :], res_sb[:, g, :], ident[:])
                nc.scalar.copy(kout[:NJ, g, :], tp[:NJ, :])
                nc.sync.dma_start(
                    kmax[g].rearrange("(j t) -> j t", t=128), kout[:NJ, g, :],
                )

    nc.compile()
    return nc


def kernel(x, Wq, bq, Wk, bk):
    global _COMPILED
    if _COMPILED is None:
        _COMPILED = _build()
    nc = _COMPILED

    x = np.ascontiguousarray(x, dtype=np.float32)
    xT = np.ascontiguousarray(
        x.reshape(B, C, HW).transpose(1, 0, 2).reshape(C, FLAT)
    )
    wqT = np.ascontiguousarray(np.asarray(Wq, np.float32).T)
    wkT = np.ascontiguousarray(np.asarray(Wk, np.float32).T)
    bq2 = np.ascontiguousarray(np.asarray(bq, np.float32).reshape(C8, 1))
    bk2 = np.ascontiguousarray(np.asarray(bk, np.float32).reshape(C8, 1))

    in_maps = [
        {
            "xr": np.ascontiguousarray(np.roll(xT, -i * GROLL, axis=1)),
            "wq": wqT,
            "wk": wkT,
            "bq": bq2,
            "bk": bk2,
        }
        for i in range(N_CORES)
    ]

    res = run_bass_kernel_spmd(nc, in_maps, core_ids=list(range(N_CORES)))

    kernel_max = np.empty((B, FLAT), np.float32)
    pos_max = np.empty((B, HW), np.float32)
    for i, r in enumerate(res.results):
        kernel_max[i * GPC:(i + 1) * GPC] = np.roll(r["kmax"], i * GROLL, axis=1)
        pos_max[i * GPC:(i + 1) * GPC] = r["pmax"]

    return np.concatenate(
        [kernel_max.reshape(B, B, HW), pos_max[None]], axis=0
    ).astype(np.float32)



# revision 16
# speedup vs baseline: 1.4513x; 1.4513x over previous
"""Trainium2 Bass kernel for nn_AlignModule (QAConv correlation + PAM).

Reference computation (B=32, C=512, H=24, W=8, hw=192, C8=64):
  xf = x.reshape(B, C, hw)
  score[g,p,n,m] = sum_c xf[g,c,m] * xf[p,c,n]          # [B,B,hw,hw]
  kernel_max[g,p,n] = max_m score[g,p,n,m]              # [B,B,hw]
  q = Wq @ xf[b] + bq; k = Wk @ xf[b] + bk              # [B,C8,hw]
  energy[b,m,n] = sum_q q[b,q,m] k[b,q,n]
  pos_max[b,m] = max_n energy[b,m,n]                    # [B,hw]
  out = concat([kernel_max, pos_max[None]], axis=0)     # [B+1,B,hw]

Sharding: data-parallel over g across 8 cores (4 images per core). Each
core gets the full x as [C, B*hw] fp8(e4m3), rolled so its own 4 images
occupy columns [0, 768).

Speed strategy vs the fp32r baseline (84.4us):
 - All QAConv/projection matmuls run in fp8(e4m3) with DoubleRow perf
   mode: one instruction consumes 256 contraction rows at 0.5 cycles
   per moving column -> 4x the fp32r matmul throughput. Tolerance
   allows it: rel err vs output absmax is ~1.3e-2 < 2e-2.
 - The max-reduction over the score (the second bottleneck, 4.7M fp32
   psum elements per core) is spread over DVE, Act and Pool. GPSIMD
   cannot touch PSUM and no engine may read two PSUM operands in one
   instruction, so the two routes are:
     D: DVE reduce_max straight from psum (fp32)
     A: Act copy/cast psum -> sbuf fp16, then DVE 4x-mode reduce_max
   (GPSIMD supports no elementwise/reduce work in this toolchain, so
   Pool only does the small memset.)
 - PSUM is two 4-bank tiles (2 j-groups each) in one rotating pool;
   the PAM projection / energy generations share the same rotation.
 - The six score tiles holding g==p diagonal blocks accumulate an
   extra fp8 hi*lo + lo*hi cross-term (PE has slack), halving the fp8
   quantization error: rel err ~6.8e-3 vs the 2e-2 gate.
 - No on-chip transposes: outputs are stored reduction-friendly and the
   host de-interleaves.
"""

import numpy as np
import ml_dtypes

import concourse.bass as bass
import concourse.mybir as mybir
import concourse.tile as tile
from concourse import bacc
from concourse.bass_utils import run_bass_kernel_spmd

B = 32
C = 512
HW = 192
C8 = 64
N_CORES = 8
GPC = B // N_CORES            # images per core (4)
FLAT = B * HW                 # flattened (p, n) axis (6144)
NJ = FLAT // 128              # stationary 128-column blocks (48)
NPAIR = NJ // 2               # psum generations of 2 j-blocks (24)
GROLL = GPC * HW              # per-core roll step (768)
NCH = FLAT // GROLL           # x column chunks (8)
JPC = GROLL // 128            # j blocks per column chunk (6)

F32 = mybir.dt.float32
F16 = mybir.dt.float16
F8 = mybir.dt.float8e4
AX_X = mybir.AxisListType.X
DR = mybir.MatmulPerfMode.DoubleRow
IDENT = mybir.ActivationFunctionType.Identity

# consumer route per pair: D = DVE direct reduce, A = Act cast + DVE
# 4x finish. 9 D / 15 A balances DVE against Act; the first emitted
# pairs (1, 2) are D so Act can do the PAM bias casts early, and the
# last pair is D for a short drain tail.
_D_AT = {1, 2, 5, 8, 11, 13, 16, 19, 23}
ROUTES = ["D" if i in _D_AT else "A" for i in range(NPAIR)]

# tiles (j, gp) that contain g==p diagonal blocks (rolled layout puts
# the core's own images at columns [0, 768) -> j 0..5)
DIAG_TILES = {(0, 0), (1, 0), (2, 0), (3, 1), (4, 1), (5, 1)}

_COMPILED = None


def _build():
    nc = bacc.Bacc("TRN2", target_bir_lowering=False, debug=False)

    xr = nc.dram_tensor("xr", [C, FLAT], F8, kind="ExternalInput").ap()
    xlo = nc.dram_tensor("xlo", [C, GROLL], F8, kind="ExternalInput").ap()
    wq = nc.dram_tensor("wq", [C, C8], F8, kind="ExternalInput").ap()
    wk = nc.dram_tensor("wk", [C, C8], F8, kind="ExternalInput").ap()
    bq = nc.dram_tensor("bq", [C8, 1], F32, kind="ExternalInput").ap()
    bk = nc.dram_tensor("bk", [C8, 1], F32, kind="ExternalInput").ap()
    kres = nc.dram_tensor("kres", [128, NJ, 2, 2], F16, kind="ExternalOutput").ap()
    pam = nc.dram_tensor("pam", [128, 2 * GPC], F16, kind="ExternalOutput").ap()

    xrr = xr.rearrange("(co p) f -> p co f", p=128)
    with tile.TileContext(nc) as tc:
        with (
            tc.tile_pool(name="sb", bufs=1) as sb,
            tc.tile_pool(name="cpool", bufs=3) as cpool,
            tc.tile_pool(name="psum", bufs=2, space="PSUM") as psum,
        ):
            # ---- input DMAs: own-image chunk first, then weights, rest ----
            xc = [None] * NCH
            for c in (0,):
                t = sb.tile([128, 4, GROLL], F8, tag=f"x{c}", name=f"x{c}")
                nc.sync.dma_start(t[:], xrr[:, :, c * GROLL:(c + 1) * GROLL])
                xc[c] = t
            xlo_sb = sb.tile([128, 4, GROLL], F8, tag="xlo", name="xlo_sb")
            nc.sync.dma_start(xlo_sb[:], xlo.rearrange("(co p) f -> p co f", p=128))
            wq_sb = sb.tile([128, 4, C8], F8, tag="wq", name="wq_sb")
            nc.sync.dma_start(wq_sb[:], wq.rearrange("(co p) q -> p co q", p=128))
            wk_sb = sb.tile([128, 4, C8], F8, tag="wk", name="wk_sb")
            nc.sync.dma_start(wk_sb[:], wk.rearrange("(co p) q -> p co q", p=128))
            bq_sb = sb.tile([C8, 1], F32, tag="bq", name="bq_sb")
            nc.sync.dma_start(bq_sb[:], bq[:])
            bk_sb = sb.tile([C8, 1], F32, tag="bk", name="bk_sb")
            nc.sync.dma_start(bk_sb[:], bk[:])
            for c in range(1, NCH):
                t = sb.tile([128, 4, GROLL], F8, tag=f"x{c}", name=f"x{c}")
                nc.sync.dma_start(t[:], xrr[:, :, c * GROLL:(c + 1) * GROLL])
                xc[c] = t

            # ---- persistent sbuf ----
            # q/k projections, fp16: [c8, {q,k}, 4*HW + 64 zero pad]
            qk_sb = sb.tile([C8, 2, GPC * HW + C8], F16, tag="qk", name="qk_sb")
            res_sb = sb.tile([128, NJ, 2, 2], F16, tag="res", name="res_sb")
            pam_sb = sb.tile([128, 2 * GPC], F16, tag="pam", name="pam_sb")

            # zero the energy stationary pad (read by the b=3 m-chunk)
            nc.gpsimd.memset(qk_sb[:, :, GPC * HW:], 0.0)

            def pair_matmuls(q):
                pt = psum.tile([128, 4, 512], F32, tag="ps", name=f"qa_{q}")
                for jl in range(2):
                    j = 2 * q + jl
                    cc, jp = divmod(j, JPC)
                    for gp in range(2):
                        out = pt[:, 2 * jl + gp, 0:2 * HW]
                        ops = [(xc[cc], xc[0])]
                        if (j, gp) in DIAG_TILES:
                            ops += [(xlo_sb, xc[0]), (xc[0], xlo_sb)]
                        n = 2 * len(ops)
                        i = 0
                        for lt, rt in ops:
                            for kt in range(2):
                                nc.tensor.matmul(
                                    out,
                                    lt[:, 2 * kt:2 * kt + 2,
                                       jp * 128:(jp + 1) * 128],
                                    rt[:, 2 * kt:2 * kt + 2,
                                       gp * 2 * HW:(gp + 1) * 2 * HW],
                                    start=(i == 0), stop=(i == n - 1),
                                    perf_mode=DR,
                                )
                                i += 1
                return pt

            def seg4(t):
                # [128, 4, 384] slice viewed as [128, 4, 2 segs, 192]
                return t[:, :, 0:2 * HW].rearrange("p b (s m) -> p b s m", s=2)

            def pair_consume(q, pt):
                ap4 = seg4(pt)
                res = res_sb[:, 2 * q:2 * q + 2, :, :].rearrange(
                    "p j g s -> p (j g) s")
                if ROUTES[q] == "D":
                    nc.vector.reduce_max(res, ap4, axis=AX_X)
                else:
                    cst = cpool.tile([128, 4, 2, HW], F16, tag="cast",
                                     name=f"cast_{q}")
                    nc.scalar.copy(cst[:], ap4)
                    nc.vector.reduce_max(res, cst[:], axis=AX_X)

            def pam_proj():
                # q gp0 -> bank0, q gp1 -> bank1, k gp0 -> bank2, k gp1 -> b3
                pt = psum.tile([128, 4, 512], F32, tag="ps", name="proj")
                for qi, w_sb in enumerate((wq_sb, wk_sb)):
                    for gp in range(2):
                        for kt in range(2):
                            nc.tensor.matmul(
                                pt[0:C8, 2 * qi + gp, 0:2 * HW],
                                w_sb[:, 2 * kt:2 * kt + 2, :],
                                xc[0][:, 2 * kt:2 * kt + 2,
                                      gp * 2 * HW:(gp + 1) * 2 * HW],
                                start=(kt == 0), stop=(kt == 1), perf_mode=DR,
                            )
                return pt

            def pam_cast(pt):
                # one Act instruction per projection (both gp halves)
                for qi, b_sb in enumerate((bq_sb, bk_sb)):
                    nc.scalar.activation(
                        qk_sb[:, qi, 0:2 * GROLL // 2].rearrange(
                            "p (g m) -> p g m", g=2),
                        pt[0:C8, 2 * qi:2 * qi + 2, 0:2 * HW],
                        IDENT, bias=b_sb[:],
                    )

            def pam_energy():
                # 8 slots (b, mch) -> bank s//2, offset (s%2)*256
                et = psum.tile([128, 4, 512], F32, tag="ps", name="energy")
                for b in range(GPC):
                    for mch in range(2):
                        s = 2 * b + mch
                        nc.tensor.matmul(
                            et[:, s // 2, (s % 2) * 256:(s % 2) * 256 + HW],
                            qk_sb[:, 0, b * HW + mch * 128:
                                  b * HW + (mch + 1) * 128],
                            qk_sb[:, 1, b * HW:(b + 1) * HW],
                            start=True, stop=True,
                        )
                return et

            def pam_reduce(et):
                ap4 = et[:, :, :].rearrange(
                    "p b (s x) -> p b s x", s=2)[:, :, :, 0:HW]
                # images 0-1 straight off psum on DVE
                nc.vector.reduce_max(
                    pam_sb[:, 0:4].rearrange("p (b s) -> p b s", b=2),
                    ap4[:, 0:2], axis=AX_X,
                )
                # images 2-3 via the Act route
                cst = cpool.tile([128, 2, 2, HW], F16, tag="cast",
                                 name="cast_pam")
                nc.scalar.copy(cst[:], ap4[:, 2:4])
                nc.vector.reduce_max(
                    pam_sb[:, 4:8].rearrange("p (b s) -> p b s", b=2),
                    cst[:], axis=AX_X,
                )

            # ---- emission schedule ----
            # pair 0 (whose diag tiles need xlo) is pushed mid-stream so
            # the PE never waits on the xlo DMA.
            order = list(range(1, 13)) + [0] + list(range(13, NPAIR))
            prev = None
            proj = energy = None
            for idx, q in enumerate(order):
                pt = pair_matmuls(q)
                if prev is not None:
                    pair_consume(*prev)
                prev = (q, pt)
                if idx == 1:
                    proj = pam_proj()
                elif idx == 2:
                    pam_cast(proj)
                elif idx == 6:
                    energy = pam_energy()
                elif idx == 8:
                    pam_reduce(energy)
                elif idx == 14:
                    # pairs 1..12 are consumed by now -> j 2..25 done
                    nc.sync.dma_start(kres[:, 2:26, :, :],
                                      res_sb[:, 2:26, :, :])

            pair_consume(*prev)
            nc.sync.dma_start(kres[:, 0:2, :, :], res_sb[:, 0:2, :, :])
            nc.sync.dma_start(kres[:, 26:NJ, :, :], res_sb[:, 26:NJ, :, :])
            nc.sync.dma_start(pam[:], pam_sb[:])

    nc.compile()
    return nc


def kernel(x, Wq, bq, Wk, bk):
    global _COMPILED
    if _COMPILED is None:
        _COMPILED = _build()
    nc = _COMPILED

    x = np.ascontiguousarray(x, dtype=np.float32)
    xT = x.reshape(B, C, HW).transpose(1, 0, 2).reshape(C, FLAT)
    xT8 = np.ascontiguousarray(xT).astype(ml_dtypes.float8_e4m3)
    xT8f = xT8.astype(np.float32)
    wq8 = np.ascontiguousarray(np.asarray(Wq, np.float32).T).astype(
        ml_dtypes.float8_e4m3)
    wk8 = np.ascontiguousarray(np.asarray(Wk, np.float32).T).astype(
        ml_dtypes.float8_e4m3)
    bq2 = np.ascontiguousarray(np.asarray(bq, np.float32).reshape(C8, 1))
    bk2 = np.ascontiguousarray(np.asarray(bk, np.float32).reshape(C8, 1))

    in_maps = [
        {
            "xr": np.ascontiguousarray(np.roll(xT8, -i * GROLL, axis=1)),
            "xlo": np.ascontiguousarray(
                xT[:, i * GROLL:(i + 1) * GROLL]
                - xT8f[:, i * GROLL:(i + 1) * GROLL]
            ).astype(ml_dtypes.float8_e4m3),
            "wq": wq8,
            "wk": wk8,
            "bq": bq2,
            "bk": bk2,
        }
        for i in range(N_CORES)
    ]

    res = run_bass_kernel_spmd(nc, in_maps, core_ids=list(range(N_CORES)))

    kernel_max = np.empty((B, FLAT), np.float32)
    pos_max = np.empty((B, HW), np.float32)
    for i, r in enumerate(res.results):
        kr = np.asarray(r["kres"]).astype(np.float32)   # [128, NJ, 2, 2]
        arr = kr.transpose(2, 3, 1, 0).reshape(GPC, FLAT)
        for gl in range(GPC):
            kernel_max[i * GPC + gl] = np.roll(arr[gl], i * GROLL)
        pm = np.asarray(r["pam"]).astype(np.float32)    # [128, 8]
        for b in range(GPC):
            pos_max[i * GPC + b, 0:128] = pm[:, 2 * b]
            pos_max[i * GPC + b, 128:HW] = pm[0:C8, 2 * b + 1]

    return np.concatenate(
        [kernel_max.reshape(B, B, HW), pos_max[None]], axis=0
    ).astype(np.float32)


# revision 21
# speedup vs baseline: 1.5555x; 1.0717x over previous
"""Trainium2 Bass kernel for nn_AlignModule (QAConv correlation + PAM).

Reference computation (B=32, C=512, H=24, W=8, hw=192, C8=64):
  xf = x.reshape(B, C, hw)
  score[g,p,n,m] = sum_c xf[g,c,m] * xf[p,c,n]          # [B,B,hw,hw]
  kernel_max[g,p,n] = max_m score[g,p,n,m]              # [B,B,hw]
  q = Wq @ xf[b] + bq; k = Wk @ xf[b] + bk              # [B,C8,hw]
  energy[b,m,n] = sum_q q[b,q,m] k[b,q,n]
  pos_max[b,m] = max_n energy[b,m,n]                    # [B,hw]
  out = concat([kernel_max, pos_max[None]], axis=0)     # [B+1,B,hw]

Sharding: data-parallel over g across 8 cores (4 images per core). Each
core gets the full x as [C, B*hw] fp8(e4m3), rolled so its own 4 images
occupy columns [0, 768).

Speed strategy vs the fp32r baseline (84.4us):
 - All QAConv/projection matmuls run in fp8(e4m3) with DoubleRow perf
   mode: one instruction consumes 256 contraction rows at 0.5 cycles
   per moving column -> 4x the fp32r matmul throughput. Tolerance
   allows it: rel err vs output absmax is ~1.3e-2 < 2e-2.
 - The max-reduction over the score (the second bottleneck, 4.7M fp32
   psum elements per core) is spread over DVE, Act and Pool. GPSIMD
   cannot touch PSUM and no engine may read two PSUM operands in one
   instruction, so the two routes are:
     D: DVE reduce_max straight from psum (fp32, no perf mode)
     A: Act copy/cast psum -> sbuf fp16, then a DVE tensor_max funnel
        tree (2x perf mode) + a final small reduce_max
   (GPSIMD supports no elementwise/reduce work in this toolchain, and
   TensorReduce/TensorTensorReduce get no DVE perf modes in the cost
   model, so the 2x tensor_max tree is the fastest drain available.)
 - PSUM is two 4-bank tiles (2 j-groups each) in one rotating pool;
   the PAM projection / energy generations share the same rotation.
 - The six score tiles holding g==p diagonal blocks accumulate an
   extra fp8 hi*lo + lo*hi cross-term (PE has slack), halving the fp8
   quantization error: rel err ~6.8e-3 vs the 2e-2 gate.
 - No on-chip transposes: outputs are stored reduction-friendly and the
   host de-interleaves.
"""

import numpy as np
import ml_dtypes

import concourse.bass as bass
import concourse.mybir as mybir
import concourse.tile as tile
from concourse import bacc
from concourse.bass_utils import run_bass_kernel_spmd

B = 32
C = 512
HW = 192
C8 = 64
N_CORES = 8
GPC = B // N_CORES            # images per core (4)
FLAT = B * HW                 # flattened (p, n) axis (6144)
NJ = FLAT // 128              # stationary 128-column blocks (48)
NPAIR = NJ // 2               # psum generations of 2 j-blocks (24)
GROLL = GPC * HW              # per-core roll step (768)
NCH = FLAT // GROLL           # x column chunks (8)
JPC = GROLL // 128            # j blocks per column chunk (6)

F32 = mybir.dt.float32
F16 = mybir.dt.float16
F8 = mybir.dt.float8e4
AX_X = mybir.AxisListType.X
DR = mybir.MatmulPerfMode.DoubleRow
IDENT = mybir.ActivationFunctionType.Identity

# consumer route per pair: D = DVE direct reduce, A = Act cast + DVE
# funnel tree. 5 D / 19 A balances DVE against Act; the last pair is D
# for a short drain tail.
_D_AT = {0, 3, 8, 13, 23}
ROUTES = ["D" if i in _D_AT else "A" for i in range(NPAIR)]

# tiles (j, gp) that contain g==p diagonal blocks (rolled layout puts
# the core's own images at columns [0, 768) -> j 0..5)
DIAG_TILES = {(0, 0), (1, 0), (2, 0), (3, 1), (4, 1), (5, 1)}

_COMPILED = None


def _build():
    nc = bacc.Bacc("TRN2", target_bir_lowering=False, debug=False)

    xr = nc.dram_tensor("xr", [C, FLAT], F8, kind="ExternalInput").ap()
    xlo = nc.dram_tensor("xlo", [C, GROLL], F8, kind="ExternalInput").ap()
    wq = nc.dram_tensor("wq", [C, C8], F8, kind="ExternalInput").ap()
    wk = nc.dram_tensor("wk", [C, C8], F8, kind="ExternalInput").ap()
    bq = nc.dram_tensor("bq", [C8, 1], F32, kind="ExternalInput").ap()
    bk = nc.dram_tensor("bk", [C8, 1], F32, kind="ExternalInput").ap()
    kres = nc.dram_tensor("kres", [128, NJ, 2, 2], F16, kind="ExternalOutput").ap()
    pam = nc.dram_tensor("pam", [128, 2 * GPC], F16, kind="ExternalOutput").ap()

    xrr = xr.rearrange("(co p) f -> p co f", p=128)
    with tile.TileContext(nc) as tc:
        with (
            tc.tile_pool(name="sb", bufs=1) as sb,
            tc.tile_pool(name="cpool", bufs=3) as cpool,
            tc.tile_pool(name="psum", bufs=2, space="PSUM") as psum,
        ):
            # ---- input DMAs: own-image chunk (split for a fast PE start),
            # then xlo + next chunk, then weights, then the rest ----
            xc = [None] * NCH
            xc[0] = sb.tile([128, 4, GROLL], F8, tag="x0", name="x0")
            nc.sync.dma_start(xc[0][:, :, 0:GROLL // 2],
                              xrr[:, :, 0:GROLL // 2])
            nc.sync.dma_start(xc[0][:, :, GROLL // 2:GROLL],
                              xrr[:, :, GROLL // 2:GROLL])
            xlo_sb = sb.tile([128, 4, GROLL], F8, tag="xlo", name="xlo_sb")
            nc.sync.dma_start(xlo_sb[:], xlo.rearrange("(co p) f -> p co f", p=128))
            xc[1] = sb.tile([128, 4, GROLL], F8, tag="x1", name="x1")
            nc.sync.dma_start(xc[1][:], xrr[:, :, GROLL:2 * GROLL])
            wq_sb = sb.tile([128, 4, C8], F8, tag="wq", name="wq_sb")
            nc.sync.dma_start(wq_sb[:], wq.rearrange("(co p) q -> p co q", p=128))
            wk_sb = sb.tile([128, 4, C8], F8, tag="wk", name="wk_sb")
            nc.sync.dma_start(wk_sb[:], wk.rearrange("(co p) q -> p co q", p=128))
            bq_sb = sb.tile([C8, 1], F32, tag="bq", name="bq_sb")
            nc.sync.dma_start(bq_sb[:], bq[:])
            bk_sb = sb.tile([C8, 1], F32, tag="bk", name="bk_sb")
            nc.sync.dma_start(bk_sb[:], bk[:])
            for c in range(2, NCH):
                t = sb.tile([128, 4, GROLL], F8, tag=f"x{c}", name=f"x{c}")
                nc.sync.dma_start(t[:], xrr[:, :, c * GROLL:(c + 1) * GROLL])
                xc[c] = t

            # ---- persistent sbuf ----
            # q/k projections, fp16: [c8, {q,k}, 4*HW + 64 zero pad]
            qk_sb = sb.tile([C8, 2, GPC * HW + C8], F16, tag="qk", name="qk_sb")
            res_sb = sb.tile([128, NJ, 2, 2], F16, tag="res", name="res_sb")
            pam_sb = sb.tile([128, 2 * GPC], F16, tag="pam", name="pam_sb")

            # zero the energy stationary pad (read by the b=3 m-chunk)
            nc.gpsimd.memset(qk_sb[:, :, GPC * HW:], 0.0)

            def pair_matmuls(q):
                pt = psum.tile([128, 4, 512], F32, tag="ps", name=f"qa_{q}")
                for jl in range(2):
                    j = 2 * q + jl
                    cc, jp = divmod(j, JPC)
                    for gp in range(2):
                        out = pt[:, 2 * jl + gp, 0:2 * HW]
                        ops = [(xc[cc], xc[0])]
                        if (j, gp) in DIAG_TILES:
                            ops += [(xlo_sb, xc[0]), (xc[0], xlo_sb)]
                        n = 2 * len(ops)
                        i = 0
                        for lt, rt in ops:
                            for kt in range(2):
                                nc.tensor.matmul(
                                    out,
                                    lt[:, 2 * kt:2 * kt + 2,
                                       jp * 128:(jp + 1) * 128],
                                    rt[:, 2 * kt:2 * kt + 2,
                                       gp * 2 * HW:(gp + 1) * 2 * HW],
                                    start=(i == 0), stop=(i == n - 1),
                                    perf_mode=DR,
                                )
                                i += 1
                return pt

            def seg4(t):
                # [128, 4, 384] slice viewed as [128, 4, 2 segs, 192]
                return t[:, :, 0:2 * HW].rearrange("p b (s m) -> p b s m", s=2)

            def tree_finish(src, out, tag):
                # src: [128, 4, 2, 192] fp16 sbuf -> out [128, 4, 2] via a
                # 2x-mode tensor_max funnel + small final reduce
                t1 = cpool.tile([128, 4, 2, 96], F16, tag="t1",
                                name=f"t1_{tag}")
                nc.vector.tensor_max(t1[:], src[:, :, :, 0:96],
                                     src[:, :, :, 96:192])
                t2 = cpool.tile([128, 4, 2, 48], F16, tag="t2",
                                name=f"t2_{tag}")
                nc.vector.tensor_max(t2[:], t1[:, :, :, 0:48],
                                     t1[:, :, :, 48:96])
                t3 = cpool.tile([128, 4, 2, 24], F16, tag="t3",
                                name=f"t3_{tag}")
                nc.vector.tensor_max(t3[:], t2[:, :, :, 0:24],
                                     t2[:, :, :, 24:48])
                nc.vector.reduce_max(out, t3[:], axis=AX_X)

            def pair_consume(q, pt):
                ap4 = seg4(pt)
                res = res_sb[:, 2 * q:2 * q + 2, :, :].rearrange(
                    "p j g s -> p (j g) s")
                if ROUTES[q] == "D":
                    nc.vector.reduce_max(res, ap4, axis=AX_X)
                else:
                    cst = cpool.tile([128, 4, 2, HW], F16, tag="cast",
                                     name=f"cast_{q}")
                    nc.scalar.copy(cst[:], ap4)
                    tree_finish(cst, res, f"q{q}")

            def pam_proj():
                # q gp0 -> bank0, q gp1 -> bank1, k gp0 -> bank2, k gp1 -> b3
                pt = psum.tile([128, 4, 512], F32, tag="ps", name="proj")
                for qi, w_sb in enumerate((wq_sb, wk_sb)):
                    for gp in range(2):
                        for kt in range(2):
                            nc.tensor.matmul(
                                pt[0:C8, 2 * qi + gp, 0:2 * HW],
                                w_sb[:, 2 * kt:2 * kt + 2, :],
                                xc[0][:, 2 * kt:2 * kt + 2,
                                      gp * 2 * HW:(gp + 1) * 2 * HW],
                                start=(kt == 0), stop=(kt == 1), perf_mode=DR,
                            )
                return pt

            def pam_cast(pt):
                # one Act instruction per projection (both gp halves)
                for qi, b_sb in enumerate((bq_sb, bk_sb)):
                    nc.scalar.activation(
                        qk_sb[:, qi, 0:2 * GROLL // 2].rearrange(
                            "p (g m) -> p g m", g=2),
                        pt[0:C8, 2 * qi:2 * qi + 2, 0:2 * HW],
                        IDENT, bias=b_sb[:],
                    )

            def pam_energy():
                # 8 slots (b, mch) -> bank s//2, offset (s%2)*256
                et = psum.tile([128, 4, 512], F32, tag="ps", name="energy")
                for b in range(GPC):
                    for mch in range(2):
                        s = 2 * b + mch
                        nc.tensor.matmul(
                            et[:, s // 2, (s % 2) * 256:(s % 2) * 256 + HW],
                            qk_sb[:, 0, b * HW + mch * 128:
                                  b * HW + (mch + 1) * 128],
                            qk_sb[:, 1, b * HW:(b + 1) * HW],
                            start=True, stop=True,
                        )
                return et

            def pam_reduce(et):
                ap4 = et[:, :, :].rearrange(
                    "p b (s x) -> p b s x", s=2)[:, :, :, 0:HW]
                cst = cpool.tile([128, 4, 2, HW], F16, tag="cast",
                                 name="cast_pam")
                nc.scalar.copy(cst[:], ap4)
                tree_finish(
                    cst, pam_sb[:].rearrange("p (b s) -> p b s", b=GPC),
                    "pam")

            # ---- emission schedule ----
            # pair 0 (whose diag tiles need xlo) is pushed mid-stream so
            # the PE never waits on the xlo DMA.
            order = list(range(1, 13)) + [0] + list(range(13, NPAIR))
            proj = energy = None
            for idx, q in enumerate(order):
                pt = pair_matmuls(q)
                pair_consume(q, pt)
                if idx == 4:
                    proj = pam_proj()
                elif idx == 5:
                    pam_cast(proj)
                elif idx == 8:
                    energy = pam_energy()
                elif idx == 10:
                    pam_reduce(energy)
                elif idx == 13:
                    # pairs 1..12 and 0 are consumed by now -> j 0..25 done
                    nc.sync.dma_start(kres[:, 0:26, :, :],
                                      res_sb[:, 0:26, :, :])

            nc.sync.dma_start(kres[:, 26:NJ, :, :], res_sb[:, 26:NJ, :, :])
            nc.sync.dma_start(pam[:], pam_sb[:])

    nc.compile()
    return nc


def kernel(x, Wq, bq, Wk, bk):
    global _COMPILED
    if _COMPILED is None:
        _COMPILED = _build()
    nc = _COMPILED

    x = np.ascontiguousarray(x, dtype=np.float32)
    xT = x.reshape(B, C, HW).transpose(1, 0, 2).reshape(C, FLAT)
    xT8 = np.ascontiguousarray(xT).astype(ml_dtypes.float8_e4m3)
    xT8f = xT8.astype(np.float32)
    wq8 = np.ascontiguousarray(np.asarray(Wq, np.float32).T).astype(
        ml_dtypes.float8_e4m3)
    wk8 = np.ascontiguousarray(np.asarray(Wk, np.float32).T).astype(
        ml_dtypes.float8_e4m3)
    bq2 = np.ascontiguousarray(np.asarray(bq, np.float32).reshape(C8, 1))
    bk2 = np.ascontiguousarray(np.asarray(bk, np.float32).reshape(C8, 1))

    in_maps = [
        {
            "xr": np.ascontiguousarray(np.roll(xT8, -i * GROLL, axis=1)),
            "xlo": np.ascontiguousarray(
                xT[:, i * GROLL:(i + 1) * GROLL]
                - xT8f[:, i * GROLL:(i + 1) * GROLL]
            ).astype(ml_dtypes.float8_e4m3),
            "wq": wq8,
            "wk": wk8,
            "bq": bq2,
            "bk": bk2,
        }
        for i in range(N_CORES)
    ]

    res = run_bass_kernel_spmd(nc, in_maps, core_ids=list(range(N_CORES)))

    kernel_max = np.empty((B, FLAT), np.float32)
    pos_max = np.empty((B, HW), np.float32)
    for i, r in enumerate(res.results):
        kr = np.asarray(r["kres"]).astype(np.float32)   # [128, NJ, 2, 2]
        arr = kr.transpose(2, 3, 1, 0).reshape(GPC, FLAT)
        for gl in range(GPC):
            kernel_max[i * GPC + gl] = np.roll(arr[gl], i * GROLL)
        pm = np.asarray(r["pam"]).astype(np.float32)    # [128, 8]
        for b in range(GPC):
            pos_max[i * GPC + b, 0:128] = pm[:, 2 * b]
            pos_max[i * GPC + b, 128:HW] = pm[0:C8, 2 * b + 1]

    return np.concatenate(
        [kernel_max.reshape(B, B, HW), pos_max[None]], axis=0
    ).astype(np.float32)
